# revision 34
# baseline (speedup 1.0000x reference)
"""Embedding lookup kernel for Trainium2 (8 NeuronCores, data-parallel).

out[b, s, :] = emb_table[road_map[data[b, s, 0]]], zeros where data == PAD_ID.

Per core (65536 ids), a 5-stage pipeline over 8 chunks of 8192 ids:
  A: SWDGE dma_gather of 256B road_map rows (row = id >> 7) from HBM,
     one descriptor per id                                      [16 MB read]
  S: DVE select of element (id & 127) from each row via
     iota == r mask, multiply, reduce-max          -> cluster id per id
  W: tensor-engine fold of cid [128, C] into the wrap-16 idx layout
     dma_gather wants ([16, CH/16] replicated x8 groups): 8 one-hot
     matmuls into PSUM + one Act-engine copy to int16
  B: SWDGE dma_gather of 256B bf16 embedding rows by cluster id [16 MB read]
  C: HWDGE store, each partition writes contiguous 16KB runs    [16 MB write]

A and B run on separate SWDGE queues; S on DVE, W on Act, C on SP, so all
five stages overlap. The embedding table is staged bf16 (rel err ~2^-8,
well under the 2e-2 gate), which also halves the result download and the
donated zero-buffer upload that dominate wall time on this runtime.

Host staging is data-independent: road_map entry PAD -> 4096 plus a zero
row appended to the table, q = id>>7 / r = id&127 bit splits, and layout
permutations so that gather position i of chunk t holds shard id
t*8192 + (i//128)*128 + (i%128) and partition p's stores are contiguous.
"""

from contextlib import ExitStack

import numpy as np
import ml_dtypes

import concourse.bacc as bacc
import concourse.bass as bass
import concourse.mybir as mybir
from concourse.bass_utils import run_bass_kernel_spmd

B, S, E = 128, 4096, 128
N_CORES = 8
B_SH = B // N_CORES              # 16 batches per core
N = B_SH * S                     # 65536 ids per core
ROUTEID_NUM = 100000
PAD_ID = ROUTEID_NUM + 1
CLUSTER_NUM = 4096
ZERO_ROW = CLUSTER_NUM

RM_W = 128                       # road_map entries per gathered row (256B int16)
RM_ROWS = (ROUTEID_NUM + 2 + RM_W - 1) // RM_W   # 782

NCH = 8                          # chunks per core
CH = N // NCH                    # 8192 ids per chunk
C = CH // 128                    # 64 gather columns per chunk
CW = CH // 16                    # 512 wrapped idx columns per chunk

_NC_CACHE = {}


def _build_bass():
    nc = bacc.Bacc("TRN2", num_swdge_queues=2)
    i16, bf16 = mybir.dt.int16, mybir.dt.bfloat16

    qw_d = nc.dram_tensor("qw", [128, N // 16], i16, kind="ExternalInput")
    r_d = nc.dram_tensor("rr", [128, N // 128], i16, kind="ExternalInput")
    rm_d = nc.dram_tensor("rm", [RM_ROWS, RM_W], i16, kind="ExternalInput")
    emb_d = nc.dram_tensor("emb", [CLUSTER_NUM + 1, E], bf16, kind="ExternalInput")
    out_d = nc.dram_tensor("out", [N, E], bf16, kind="ExternalOutput")
    # DRAM row p*512 + t*64 + c <- SBUF rows[t][p, c, :]
    out_v = out_d[:, :].rearrange("(p t c) e -> t p (c e)", p=128, t=NCH, c=C)

    with ExitStack() as ctx:
        sb = lambda n, s, d: ctx.enter_context(nc.sbuf_tensor(n, s, d))
        sem = lambda n: ctx.enter_context(nc.semaphore(n))

        f32 = mybir.dt.float32
        qw_sb = sb("qw_sb", [128, N // 16], i16)
        r_sb = sb("r_sb", [128, N // 128], i16)
        iota_sb = sb("iota_sb", [128, C * RM_W], i16)
        rmrow = [sb(f"rmrow{k}", [128, C * RM_W], i16) for k in range(2)]
        mask = sb("mask", [128, C * RM_W], i16)
        prod = sb("prod", [128, C * RM_W], i16)
        cid = [sb(f"cid{k}", [128, C], f32) for k in range(2)]
        cidw = [sb(f"cidw{k}", [128, CW], i16) for k in range(2)]
        rows = [sb(f"rows{k}", [128, C * E], bf16) for k in range(2)]
        ones = sb("ones", [128, 1], f32)
        wsel = sb("wsel", [128, 8 * 128], f32)   # fold weights W_d[k, m]
        psum = [
            ctx.enter_context(nc.psum_tensor(f"ps{k}", [128, CW], f32))
            for k in range(2)
        ]

        sIn, sIota, sS, sW = sem("sIn"), sem("sIota"), sem("sS"), sem("sW")
        sEq, sMul, sM, sWt = sem("sEq"), sem("sMul"), sem("sM"), sem("sWt")
        # per-parity sems: DMA completions on one sem are indistinguishable
        # by count, so each double-buffer slot gets its own semaphore
        sA = [sem("sA0"), sem("sA1")]
        sB = [sem("sB0"), sem("sB1")]
        sC = [sem("sC0"), sem("sC1")]

        iota3 = iota_sb[:, :].rearrange("p (c k) -> p c k", k=RM_W)
        rm3 = [t[:, :].rearrange("p (c k) -> p c k", k=RM_W) for t in rmrow]
        mask3 = mask[:, :].rearrange("p (c k) -> p c k", k=RM_W)
        prod3 = prod[:, :].rearrange("p (c k) -> p c k", k=RM_W)
        rows3 = [t[:, :].rearrange("p (c e) -> p c e", e=E) for t in rows]
        wsel3 = wsel[:, :].rearrange("p (d m) -> p d m", d=8)
        wsel4 = wsel[:, :].rearrange("p (d mo mi) -> p d mo mi", d=8, mo=8)

        # ---- sync engine (SP): input loads + output stores ----
        nc.sync.dma_start(qw_sb[:, :], qw_d[:, :]).then_inc(sIn, 16)
        nc.sync.dma_start(r_sb[:, :], r_d[:, :]).then_inc(sIn, 16)
        for t in range(NCH):
            nc.sync.wait_ge(sB[t % 2], 256 * (t // 2 + 1))
            nc.sync.dma_start(out_v[t], rows[t % 2][:, :]).then_inc(sC[t % 2], 16)

        # ---- gpsimd (Pool): constants, then A/B gathers interleaved ----
        nc.gpsimd.iota(iota3, pattern=[[0, C], [1, RM_W]], base=0,
                       channel_multiplier=0).then_inc(sIota, 1)
        nc.gpsimd.memset(ones[:, :], 1.0).then_inc(sWt, 1)
        nc.gpsimd.wait_ge(sWt, 1)
        # W_d[k, mo*16+mi] = 1 iff k == 16d + mi (partition-fold selectors)
        for d in range(8):
            nc.gpsimd.affine_select(
                out=wsel4[:, d, :, :],
                in_=ones[:, :].to_broadcast([128, 8, 16]),
                pattern=[[0, 8], [-1, 16]],
                compare_op=mybir.AluOpType.is_equal,
                fill=0.0,
                base=-16 * d,
                channel_multiplier=1,
            ).then_inc(sWt, 1)
        nc.gpsimd.wait_ge(sIn, 32)

        # SWDGE carveout ring holds ~64 descriptors per DMA engine; one call
        # of SC idxs needs SC/16+1, so SC=512 (33) fits with slack while
        # SC>=1024 wedges the ring (illegal_instruction on HW).
        SC = 512
        SUB = CH // SC              # 16 sub-calls per chunk
        SCC = SC // 128             # 4 gather columns per sub-call
        SCW = SC // 16              # 32 idx columns per sub-call

        def emit_A(t):
            if t >= 2:
                nc.gpsimd.wait_ge(sS, t - 1)   # rmrow[t%2] free after S(t-2)
            for s in range(SUB):
                nc.gpsimd.dma_gather(
                    rm3[t % 2][:, s * SCC:(s + 1) * SCC, :],
                    rm_d[:, :],
                    qw_sb[:, t * CW + s * SCW:t * CW + (s + 1) * SCW],
                    num_idxs=SC,
                    num_idxs_reg=SC,
                    elem_size=RM_W,
                    queue_num=0,
                ).then_inc(sA[t % 2], 16)

        def emit_B(t):
            nc.gpsimd.wait_ge(sW, t + 1)       # cidw[t%2] written
            if t >= 2:
                nc.gpsimd.wait_ge(sC[t % 2], 16 * (t // 2))  # rows[t%2] stored
            for s in range(SUB):
                nc.gpsimd.dma_gather(
                    rows3[t % 2][:, s * SCC:(s + 1) * SCC, :],
                    emb_d[:, :],
                    cidw[t % 2][:, s * SCW:(s + 1) * SCW],
                    num_idxs=SC,
                    num_idxs_reg=SC,
                    elem_size=E,
                    queue_num=1,
                ).then_inc(sB[t % 2], 16)

        emit_A(0)
        emit_A(1)
        for t in range(NCH):
            emit_B(t)
            if t + 2 < NCH:
                emit_A(t + 2)

        # ---- vector engine (DVE): select cid from gathered rows ----
        nc.vector.wait_ge(sIn, 32)
        nc.vector.wait_ge(sIota, 1)
        for t in range(NCH):
            rv = r_sb[:, t * C:(t + 1) * C].to_broadcast([128, C, RM_W])
            if t >= 1:
                nc.vector.wait_ge(sMul, t)     # mask free (mult(t-1) done)
            nc.vector.tensor_tensor(
                out=mask3, in0=iota3, in1=rv, op=mybir.AluOpType.is_equal,
            ).then_inc(sEq, 1)
            nc.vector.wait_ge(sA[t % 2], 256 * (t // 2 + 1))
            nc.vector.wait_ge(sEq, t + 1)
            if t >= 1:
                nc.vector.wait_ge(sS, t)       # prod free (reduce(t-1) done)
            nc.vector.tensor_tensor(
                out=prod3, in0=rm3[t % 2], in1=mask3, op=mybir.AluOpType.mult,
            ).then_inc(sMul, 1)
            nc.vector.wait_ge(sMul, t + 1)
            if t >= 2:
                nc.vector.wait_ge(sM, 8 * (t - 1))  # cid[t%2] free (PE read)
            nc.vector.tensor_reduce(
                out=cid[t % 2][:, :], in_=prod3,
                axis=mybir.AxisListType.X, op=mybir.AluOpType.max,
            ).then_inc(sS, 1)

        # ---- tensor engine (PE): fold cid[16d+q, c] -> psum[16g+q, d*C+c]
        # via 8 one-hot matmuls; output replicates across groups for free ----
        nc.tensor.wait_ge(sWt, 9)
        for t in range(NCH):
            nc.tensor.wait_ge(sS, t + 1)
            if t >= 2:
                nc.tensor.wait_ge(sW, t - 1)   # psum[t%2] free (copy(t-2))
            for d in range(8):
                nc.tensor.matmul(
                    out=psum[t % 2][:, d * C:(d + 1) * C],
                    lhsT=wsel3[:, d, :],
                    rhs=cid[t % 2][:, :],
                    start=True,
                    stop=True,
                ).then_inc(sM, 1)

        # ---- scalar engine (Act): psum -> int16 wrap16 idx buffer ----
        for t in range(NCH):
            nc.scalar.wait_ge(sM, 8 * (t + 1))
            if t >= 2:
                nc.scalar.wait_ge(sB[t % 2], 256 * (t // 2))  # B(t-2) read cidw
            nc.scalar.copy(
                out=cidw[t % 2][:, :], in_=psum[t % 2][:, :],
            ).then_inc(sW, 1)
    return nc


def _stage_inputs(data, road_map, emb_table):
    data = np.asarray(data).reshape(B, S)
    road_map = np.asarray(road_map, dtype=np.int32)
    emb_table = np.asarray(emb_table, dtype=np.float32)

    rm2 = road_map.copy()
    rm2[PAD_ID] = ZERO_ROW
    rm_rows = np.zeros(RM_ROWS * RM_W, np.int16)
    rm_rows[: rm2.size] = rm2.astype(np.int16)
    rm_rows = rm_rows.reshape(RM_ROWS, RM_W)

    emb2 = np.concatenate([emb_table, np.zeros((1, E), np.float32)], axis=0)
    emb2 = emb2.astype(ml_dtypes.bfloat16)

    in_maps = []
    for c in range(N_CORES):
        shard = data[c * B_SH:(c + 1) * B_SH].reshape(-1).astype(np.int32)
        q = (shard >> 7).astype(np.int16)
        r = (shard & 127).astype(np.int16)
        # B-position i = (d*C + cc)*16 + w of chunk t (id n = t*CH + i) is
        # gathered at A-position a = cc*128 + 16d + w:
        #   qw[w (+16g), t*CW + cc*8 + d] = q[n]
        #   rr[16d + w,  t*C + cc]        = r[n]
        qv = q.reshape(NCH, 8, C, 16)                  # [t, d, cc, w]
        qw = qv.transpose(3, 0, 2, 1).reshape(16, N // 16)
        qw = np.ascontiguousarray(np.tile(qw, (8, 1)))
        rr = np.ascontiguousarray(
            r.reshape(NCH, 8, C, 16).transpose(1, 3, 0, 2).reshape(128, N // 128)
        )
        in_maps.append({"qw": qw, "rr": rr, "rm": rm_rows, "emb": emb2})
    return in_maps


def _unstage_output(core_out):
    # DRAM row p*512 + t*64 + c holds shard id t*8192 + c*128 + p
    return (
        core_out.reshape(128, NCH, C, E)
        .transpose(1, 2, 0, 3)
        .reshape(B_SH, S, E)
        .astype(np.float32)
    )


def kernel(data, road_map, emb_table, trace=False, **run_kwargs):
    if "nc" not in _NC_CACHE:
        nc = _build_bass()
        nc.finalize()   # Bacc: run compile passes (reg alloc, library loads)
        _NC_CACHE["nc"] = nc
    nc = _NC_CACHE["nc"]
    in_maps = _stage_inputs(data, road_map, emb_table)
    import time

    t0 = time.time()
    res = run_bass_kernel_spmd(
        nc, in_maps, core_ids=list(range(N_CORES)), trace=trace, **run_kwargs
    )
    _NC_CACHE["spmd_wall_ns"] = int((time.time() - t0) * 1e9)
    out = np.empty((B, S, E), np.float32)
    for c in range(N_CORES):
        out[c * B_SH:(c + 1) * B_SH] = _unstage_output(
            np.asarray(res.results[c]["out"])
        )
    _NC_CACHE["last_result"] = res
    return out
